# revision 6
# baseline (speedup 1.0000x reference)
"""BERT-base + CRF Viterbi decode on 8 Trainium2 NeuronCores.

Strategy: data-parallel over batch (1 sequence per core). Activations are kept
transposed [H, S] so every weight matrix is consumed in (a pre-tiled copy of)
its natural [in, out] layout as the stationary matmul operand. LayerNorm
reductions over H (the partition dim) use ones-vector matmuls; softmax runs
without max-subtraction (attention logits are provably small for this model);
the CRF Viterbi forward scan / backtrack run sequentially on the vector engine
in exact reference FP order.

kernel(**inputs) takes the full unsharded inputs and returns (score, path)
matching reference.reference().
"""
import sys
import numpy as np

sys.path.insert(0, "/opt/trn_rl_repo")

N_TAGS = 9
START, STOP, NEG = 7, 8, -10000.0
B, S, H, NL, NH, FF = 8, 256, 768, 12, 12, 3072
DH = H // NH          # 64
NK = H // 128         # 6 k-tiles over H
NM1 = FF // 128       # 24 m-tiles over FF
EPS = 1e-12

TRACE = False         # set by test harness to collect NTFF profile
LAST = {}             # filled with exec_time_ns / trace info when TRACE

_CACHE = {}


# ---------------------------------------------------------------- device code

ACT_BUFS = {"xres": 7, "sq": 3, "xT": 8, "qT": 6, "kT": 6, "vS": 2, "expT": 4,
            "ctxT": 6, "h1": 24, "lntmp": 2, "row": 8, "e": 2, "tiny": 3,
            "tiny2": 2, "tmp9": 1, "nv81": 2, "fvt": 6, "woh": 1, "featf": 1, "fvf": 1,
            "bpf": 1, "pathf": 1, "pathi": 1}
W_BUFS = {"w768": 5, "w3072": 2, "brow": 10, "b1c": 2, "lncols": 2}
PSUM_BUFS = {"mm": 3, "ctx": 2, "stat": 2}
_BUFS = {}
_BUFS.update(ACT_BUFS); _BUFS.update(W_BUFS); _BUFS.update(PSUM_BUFS)


def _tile(pool, shape, dt, tag):
    return pool.tile(shape, dt, tag=tag, bufs=_BUFS.get(tag, 1), name=tag)


def _mybir():
    import concourse.mybir as mybir
    return mybir


def emit_layer_norm(nc, tc, pools, src, s_col, b_col, consts, out_tag):
    """Partition-dim (H) layernorm on 6 [128,256] tiles -> new tiles."""
    mybir = _mybir()
    f32 = mybir.dt.float32
    acts, psum = pools["acts"], pools["psum"]
    n = len(src)
    NTOK = src[0].shape[1]
    inv_n = 1.0 / (128 * n)

    ps_sum = _tile(psum, [1, NTOK], f32, "stat")
    ps_sq = _tile(psum, [1, NTOK], f32, "stat")
    for m in range(n):
        nc.tensor.matmul(ps_sum[:, :], consts["ones_col"][:, :], src[m][:, :],
                         start=(m == 0), stop=(m == n - 1))
    for m in range(n):
        sq = _tile(acts, [128, NTOK], f32, "sq")
        nc.scalar.square(sq[:, :], src[m][:, :])
        nc.tensor.matmul(ps_sq[:, :], consts["ones_col"][:, :], sq[:, :],
                         start=(m == 0), stop=(m == n - 1))

    row = lambda: _tile(acts, [1, NTOK], f32, "row")
    mean = row(); nc.vector.tensor_scalar_mul(mean[:, :], ps_sum[:, :], inv_n)
    ms = row(); nc.vector.tensor_scalar_mul(ms[:, :], ps_sq[:, :], inv_n)
    m2 = row(); nc.vector.tensor_mul(m2[:, :], mean[:, :], mean[:, :])
    ve = row()
    # ve = ms - m2 + eps  (variance + eps)
    nc.vector.tensor_sub(ve[:, :], ms[:, :], m2[:, :])
    nc.vector.tensor_scalar_add(ve[:, :], ve[:, :], EPS)
    sd = row(); nc.scalar.sqrt(sd[:, :], ve[:, :])
    r0 = row(); nc.vector.reciprocal(r0[:, :], sd[:, :])
    # one Newton step for rsqrt: r = r0 * (1.5 - 0.5 * ve * r0^2)
    t = row()
    nc.vector.tensor_mul(t[:, :], r0[:, :], r0[:, :])
    nc.vector.tensor_mul(t[:, :], t[:, :], ve[:, :])
    nc.vector.tensor_scalar(t[:, :], t[:, :], -0.5, 1.5,
                            mybir.AluOpType.mult, mybir.AluOpType.add)
    rstd = row(); nc.vector.tensor_mul(rstd[:, :], r0[:, :], t[:, :])

    ps_mb = _tile(psum, [128, NTOK], f32, "ctx")
    ps_rb = _tile(psum, [128, NTOK], f32, "ctx")
    nc.tensor.matmul(ps_mb[:, :], consts["ones_row"][0:1, 0:128], mean[:, :],
                     start=True, stop=True)
    nc.tensor.matmul(ps_rb[:, :], consts["ones_row"][0:1, 0:128], rstd[:, :],
                     start=True, stop=True)

    out = []
    for m in range(n):
        tmp = _tile(acts, [128, NTOK], f32, "lntmp")
        nc.vector.tensor_sub(tmp[:, :], src[m][:, :], ps_mb[:, :])
        nc.vector.tensor_mul(tmp[:, :], tmp[:, :], ps_rb[:, :])
        o = _tile(acts, [128, NTOK], f32, out_tag)
        nc.vector.tensor_scalar(o[:, :], tmp[:, :], s_col(m), b_col(m),
                                mybir.AluOpType.mult, mybir.AluOpType.add)
        out.append(o)
    return out


def emit_proj(nc, tc, pools, wdram, l, xT, bias_row, out_tag, act=None,
              b1_col=None, resid=None):
    """out^T[m] = W.T @ x^T (+bias) for m in 0..5; W streamed m-major."""
    mybir = _mybir()
    f32 = mybir.dt.float32
    acts, psum, wpool = pools["acts"], pools["psum"], pools["wpool"]
    consts = pools["consts"]
    nk = len(xT)
    NTOK = xT[0].shape[1]
    out = []
    for m in range(6):
        wt = _tile(wpool, [128, nk * 128], f32, "w768" if nk == 6 else "w3072")
        nc.sync.dma_start(out=wt[:, :], in_=wdram[l, m])
        ps = _tile(psum, [128, NTOK], f32, "mm")
        for k in range(nk):
            nc.tensor.matmul(ps[:, :], wt[:, 128 * k:128 * (k + 1)], xT[k][:, :],
                             start=(k == 0), stop=False)
        nc.tensor.matmul(ps[:, :], bias_row[0:1, 128 * m:128 * (m + 1)],
                         consts["ones_row"][0:1, 0:NTOK], start=False, stop=True)
        o = _tile(acts, [128, NTOK], f32, out_tag)
        if resid is not None:
            nc.vector.tensor_add(o[:, :], ps[:, :], resid[m][:, :])
        else:
            nc.vector.tensor_copy(o[:, :], ps[:, :])
        out.append(o)
    return out


def emit_attention(nc, tc, pools, l, xT, wq, wk, wv, brows, consts):
    """Full self-attention block; returns ctx^T tiles [6 x (128,256)]."""
    mybir = _mybir()
    f32 = mybir.dt.float32
    acts, psum, wpool = pools["acts"], pools["psum"], pools["wpool"]
    NTOK = xT[0].shape[1]

    qT = emit_proj(nc, tc, pools, wq, l, xT, brows[0], "qT")
    kT = emit_proj(nc, tc, pools, wk, l, xT, brows[1], "kT")

    # v in [S, H] layout: v = x @ Wv + bv  (Wv streamed as k-rows, natural)
    ps_v = []
    for st in range(2):
        for half in range(2):
            tag = "mm" if len(ps_v) < 2 else "ctx"
            ps_v.append(_tile(psum, [128, 384], f32, tag))
    for k in range(NK):
        wvt = _tile(wpool, [128, H], f32, "w768")
        nc.sync.dma_start(out=wvt[:, :], in_=wv[l, 128 * k:128 * (k + 1), :])
        for st in range(2):
            for half in range(2):
                nc.tensor.matmul(
                    ps_v[st * 2 + half][:, :],
                    xT[k][:, 128 * st:128 * (st + 1)],
                    wvt[:, 384 * half:384 * (half + 1)],
                    start=(k == 0), stop=False)
    vS = []
    for st in range(2):
        v = _tile(acts, [128, H], f32, "vS")
        for half in range(2):
            nc.tensor.matmul(ps_v[st * 2 + half][:, :],
                             consts["ones_row"][0:1, 0:128],
                             brows[2][0:1, 384 * half:384 * (half + 1)],
                             start=False, stop=True)
            nc.vector.tensor_copy(v[:, 384 * half:384 * (half + 1)],
                                  ps_v[st * 2 + half][:, :])
        vS.append(v)

    ctxT = [None] * 6
    ps_ctx = None
    for h in range(NH):
        mt, off = h // 2, 64 * (h % 2)
        exps = []
        for kt in range(2):
            ps_s = _tile(psum, [128, NTOK], f32, "mm")
            nc.tensor.matmul(ps_s[:, :],
                             kT[mt][off:off + 64, 128 * kt:128 * (kt + 1)],
                             qT[mt][off:off + 64, :], start=True, stop=True)
            ex = _tile(acts, [128, NTOK], f32, "expT")
            nc.scalar.activation(ex[:, :], ps_s[:, :],
                                 mybir.ActivationFunctionType.Exp, scale=0.125)
            exps.append(ex)
        ps_den = _tile(psum, [1, NTOK], f32, "stat")
        for kt in range(2):
            nc.tensor.matmul(ps_den[:, :], consts["ones_col"][:, :],
                             exps[kt][:, :], start=(kt == 0), stop=(kt == 1))
        rrow = _tile(acts, [1, NTOK], f32, "row")
        nc.vector.reciprocal(rrow[:, :], ps_den[:, :])
        ps_rb = _tile(psum, [128, NTOK], f32, "stat")
        nc.tensor.matmul(ps_rb[:, :], consts["ones_row"][0:1, 0:128],
                         rrow[:, :], start=True, stop=True)
        for kt in range(2):
            nc.vector.tensor_mul(exps[kt][:, :], exps[kt][:, :], ps_rb[:, :])
        if off == 0:
            ps_ctx = _tile(psum, [128, NTOK], f32, "ctx")
            ctxT[mt] = _tile(acts, [128, NTOK], f32, "ctxT")
        for kt in range(2):
            nc.tensor.matmul(ps_ctx[off:off + 64, :],
                             vS[kt][:, 64 * h:64 * (h + 1)], exps[kt][:, :],
                             start=(kt == 0), stop=(kt == 1))
        nc.vector.tensor_copy(ctxT[mt][off:off + 64, :], ps_ctx[off:off + 64, :])
    return ctxT


def emit_ffn(nc, tc, pools, l, xT, w1, w2, b1c_tile, brow_b2, consts):
    mybir = _mybir()
    f32 = mybir.dt.float32
    acts, psum, wpool = pools["acts"], pools["psum"], pools["wpool"]
    NTOK = xT[0].shape[1]
    h1 = []
    for mt in range(NM1):
        wt = _tile(wpool, [128, H], f32, "w768")
        nc.sync.dma_start(out=wt[:, :], in_=w1[l, mt])
        ps = _tile(psum, [128, NTOK], f32, "mm")
        for k in range(NK):
            nc.tensor.matmul(ps[:, :], wt[:, 128 * k:128 * (k + 1)], xT[k][:, :],
                             start=(k == 0), stop=(k == NK - 1))
        o = _tile(acts, [128, NTOK], f32, "h1")
        nc.scalar.activation(o[:, :], ps[:, :],
                             mybir.ActivationFunctionType.Gelu,
                             bias=b1c_tile[:, mt:mt + 1], scale=1.0)
        h1.append(o)
    out = []
    for m in range(6):
        wt = _tile(wpool, [128, FF], f32, "w3072")
        nc.sync.dma_start(out=wt[:, :], in_=w2[l, m])
        ps = _tile(psum, [128, NTOK], f32, "mm")
        for k in range(NM1):
            nc.tensor.matmul(ps[:, :], wt[:, 128 * k:128 * (k + 1)], h1[k][:, :],
                             start=(k == 0), stop=False)
        nc.tensor.matmul(ps[:, :], brow_b2[0:1, 128 * m:128 * (m + 1)],
                         consts["ones_row"][0:1, 0:NTOK], start=False, stop=True)
        o = _tile(acts, [128, NTOK], f32, "xres")
        nc.vector.tensor_add(o[:, :], ps[:, :], xT[m][:, :])
        out.append(o)
    return out


def emit_viterbi(nc, tc, pools, e_tiles, consts, L, out_score, out_path,
                 fvd, bpd, featd):
    """Sequential Viterbi on DVE. e_tiles: 2 x [128, 9] emissions (tokens
    0..255 on partitions); decode tokens 1..L."""
    mybir = _mybir()
    f32 = mybir.dt.float32
    acts = pools["acts"]
    AL = mybir.AluOpType
    X = mybir.AxisListType.X
    v = nc.vector

    # token t (1-based 1..L) -> flat feats [1, 9L] via DRAM bounce
    nc.sync.dma_start(out=featd[0:9 * 127].rearrange("(a b) -> a b", a=127),
                      in_=e_tiles[0][1:128, :])
    nc.sync.dma_start(out=featd[9 * 127:9 * L].rearrange("(a b) -> a b", a=L - 127),
                      in_=e_tiles[1][0:L - 127, :])
    FEAT = _tile(acts, [1, 9 * L], f32, "featf")
    nc.sync.dma_start(out=FEAT[0:1, :], in_=featd[0:9 * L].rearrange("(o n) -> o n", o=1))

    FV = _tile(acts, [1, 9 * (L + 1)], f32, "fvf")
    v.tensor_copy(FV[0:1, 0:9], consts["misc"][0:1, 9:18])  # fv0
    trf = consts["misc"][0:1, 27:108].rearrange("p (a b) -> p a b", a=9)
    tmp9 = _tile(acts, [1, 16], f32, "tmp9")
    for t in range(L):
        nv = _tile(acts, [1, 81], f32, "nv81")
        fv_b = FV[0:1, 9 * t:9 * (t + 1)].rearrange(
            "p (o n) -> p o n", o=1).broadcast_to([1, 9, 9])
        v.tensor_tensor(nv[0:1, :].rearrange("p (a b) -> p a b", a=9),
                        fv_b, trf, AL.add)
        v.tensor_reduce(tmp9[0:1, 0:9],
                        nv[0:1, :].rearrange("p (a b) -> p a b", a=9), X, AL.max)
        v.tensor_add(FV[0:1, 9 * (t + 1):9 * (t + 2)], tmp9[0:1, 0:9],
                     FEAT[0:1, 9 * t:9 * (t + 1)])

    # termination
    term = _tile(acts, [1, 16], f32, "tiny")
    v.tensor_add(term[0:1, 0:9], FV[0:1, 9 * L:9 * (L + 1)],
                 consts["misc"][0:1, 18:27])
    sc = _tile(acts, [1, 2], f32, "tiny2")
    v.tensor_reduce(sc[0:1, 0:1], term[0:1, 0:9], X, AL.max)
    eq = _tile(acts, [1, 16], f32, "tiny")
    v.tensor_scalar(eq[0:1, 0:9], term[0:1, 0:9], sc[0:1, 0:1], None, AL.is_equal)
    v.tensor_mul(eq[0:1, 0:9], eq[0:1, 0:9], consts["misc"][0:1, 109:118])
    rv = _tile(acts, [1, 2], f32, "tiny2")
    v.tensor_reduce(rv[0:1, 0:1], eq[0:1, 0:9], X, AL.max)

    PATHF = _tile(acts, [1, L], f32, "pathf")
    v.tensor_scalar(PATHF[0:1, L - 1:L], rv[0:1, 0:1], -1.0, 9.0, AL.mult, AL.add)

    # backpointer extraction, vectorized over t
    nc.sync.dma_start(out=fvd[0:9 * L].rearrange("(o n) -> o n", o=1), in_=FV[0:1, 0:9 * L])
    trrep = consts["trrep"]
    revj = consts["revj"]
    splits = [(0, 127), (127, L)]
    for lo, hi in splits:
        nr = hi - lo
        fvt = _tile(acts, [128, 9], f32, "fvt")
        nc.sync.dma_start(out=fvt[0:nr, :],
                          in_=fvd[9 * lo:9 * hi].rearrange("(a b) -> a b", a=nr))
        nvt = _tile(acts, [128, 81], f32, "nvt")
        nvt_v = nvt[0:nr, :].rearrange("p (a b) -> p a b", a=9)
        v.tensor_tensor(
            nvt_v,
            fvt[0:nr, :].rearrange("p (o n) -> p o n", o=1).broadcast_to([nr, 9, 9]),
            trrep[0:nr, :].rearrange("p (a b) -> p a b", a=9), AL.add)
        vt = _tile(acts, [128, 9], f32, "fvt")
        v.tensor_reduce(vt[0:nr, :], nvt_v, X, AL.max)
        v.tensor_tensor(
            nvt_v, nvt_v,
            vt[0:nr, :].rearrange("p (n o) -> p n o", o=1).broadcast_to([nr, 9, 9]),
            AL.is_equal)
        v.tensor_tensor(nvt_v, nvt_v,
                        revj[0:nr, :].rearrange("p (a b) -> p a b", a=9), AL.mult)
        rmt = _tile(acts, [128, 9], f32, "fvt")
        v.tensor_reduce(rmt[0:nr, :], nvt_v, X, AL.max)
        bpt = _tile(acts, [128, 9], f32, "fvt")
        v.tensor_scalar(bpt[0:nr, :], rmt[0:nr, :], -1.0, 9.0, AL.mult, AL.add)
        nc.sync.dma_start(out=bpd[9 * lo:9 * hi].rearrange("(a b) -> a b", a=nr),
                          in_=bpt[0:nr, :])
    BPF = _tile(acts, [1, 9 * L], f32, "bpf")
    nc.sync.dma_start(out=BPF[0:1, :], in_=bpd[0:9 * L].rearrange("(o n) -> o n", o=1))

    # backtrack
    w = _tile(acts, [1, 9], f32, "woh")
    iota9 = consts["misc"][0:1, 0:9]
    v.tensor_scalar(w[0:1, :], iota9, PATHF[0:1, L - 1:L], None, AL.is_equal)
    for t in range(L - 1, 0, -1):
        v.tensor_mul(tmp9[0:1, 0:9], w[0:1, :], BPF[0:1, 9 * t:9 * (t + 1)])
        v.tensor_reduce(PATHF[0:1, t - 1:t], tmp9[0:1, 0:9], X, AL.add)
        v.tensor_scalar(w[0:1, :], iota9, PATHF[0:1, t - 1:t], None, AL.is_equal)

    PATHI = _tile(acts, [1, L], mybir.dt.int32, "pathi")
    v.tensor_copy(PATHI[0:1, :], PATHF[0:1, :])
    nc.sync.dma_start(out=out_path[0:1, :], in_=PATHI[0:1, :])
    nc.sync.dma_start(out=out_score[0:1, :], in_=sc[0:1, 0:1])


def build(L=254, n_layers=NL):
    """Build + compile the per-core program. Returns (nc, meta)."""
    import concourse.bass as bass  # noqa
    import concourse.mybir as mybir
    from concourse import bacc
    from concourse.tile import TileContext

    f32 = mybir.dt.float32
    nc = bacc.Bacc("TRN2")
    dp = lambda name, shape, dt=f32, out=False: nc.declare_dram_parameter(
        name, shape, dt, isOutput=out)

    x0T = dp("x0T", [H, S])
    wq = dp("wq", [NL, 6, 128, H]); wk = dp("wk", [NL, 6, 128, H])
    wv = dp("wv", [NL, H, H]); wo = dp("wo", [NL, 6, 128, H])
    w1 = dp("w1", [NL, NM1, 128, H]); w2 = dp("w2", [NL, 6, 128, FF])
    brows_d = dp("brows", [NL, 5, H])           # bq bk bv bo b2
    b1c_d = dp("b1c", [NL, 128, NM1])
    lnc_d = dp("lnc", [NL, 128, 4 * NK])        # ln1s ln1b ln2s ln2b cols
    elnc_d = dp("elnc", [128, 2 * NK])
    fcw_d = dp("fcw", [128, NK, N_TAGS])
    fcb_d = dp("fcb", [1, N_TAGS])
    trrep_d = dp("c_trrep", [128, 81])
    revj_d = dp("c_revj", [128, 81])
    ones_col_d = dp("c_ones", [128, 1])
    ones_row_d = dp("c_onesrow", [1, 512])
    misc_d = dp("c_misc", [1, 128])
    out_score = dp("out_score", [1, 1], out=True)
    out_path = dp("out_path", [1, L], mybir.dt.int32, out=True)

    fvd = nc.dram_tensor("fvd", [9 * (L + 1)], f32)
    bpd = nc.dram_tensor("bpd", [9 * (L + 1)], f32)
    featd = nc.dram_tensor("featd", [9 * (L + 1)], f32)

    with TileContext(nc) as tc:
        with tc.tile_pool(name="acts", bufs=1) as acts, \
             tc.tile_pool(name="wpool", bufs=1) as wpool, \
             tc.tile_pool(name="cpool", bufs=1) as cpool, \
             tc.tile_pool(name="psum", bufs=1, space="PSUM") as psum:

            # constants (loaded once)
            consts = {}
            consts["trrep"] = _tile(cpool, [128, 81], f32, "c1")
            nc.sync.dma_start(out=consts["trrep"][:, :], in_=trrep_d[:, :])
            consts["revj"] = _tile(cpool, [128, 81], f32, "c2")
            nc.sync.dma_start(out=consts["revj"][:, :], in_=revj_d[:, :])
            consts["ones_col"] = _tile(cpool, [128, 1], f32, "c3")
            nc.sync.dma_start(out=consts["ones_col"][:, :], in_=ones_col_d[:, :])
            consts["ones_row"] = _tile(cpool, [1, 512], f32, "c4")
            nc.sync.dma_start(out=consts["ones_row"][:, :], in_=ones_row_d[:, :])
            consts["misc"] = _tile(cpool, [1, 128], f32, "c5")
            nc.sync.dma_start(out=consts["misc"][:, :], in_=misc_d[:, :])
            s_fcw = _tile(cpool, [128, NK * N_TAGS], f32, "c6")
            nc.sync.dma_start(
                out=s_fcw[:, :].rearrange("p (k n) -> p k n", k=NK),
                in_=fcw_d[:, :, :])
            s_fcb = _tile(cpool, [1, N_TAGS], f32, "c7")
            nc.sync.dma_start(out=s_fcb[:, :], in_=fcb_d[:, :])
            s_eln = _tile(cpool, [128, 2 * NK], f32, "c8")
            nc.sync.dma_start(out=s_eln[:, :], in_=elnc_d[:, :])

            pools = {"acts": acts, "psum": psum, "wpool": wpool,
                     "cpool": cpool, "consts": consts}

            # ---- embedding LN
            x0 = []
            for k in range(NK):
                t = _tile(acts, [128, S], f32, "xres")
                nc.sync.dma_start(out=t[:, :], in_=x0T[128 * k:128 * (k + 1), :])
                x0.append(t)
            xT = emit_layer_norm(nc, tc, pools, x0,
                                 lambda m: s_eln[:, m:m + 1],
                                 lambda m: s_eln[:, NK + m:NK + m + 1],
                                 consts, "xT")

            # ---- encoder layers
            for l in range(n_layers):
                brows = []
                for i in range(5):
                    bt = _tile(wpool, [1, H], f32, "brow")
                    nc.sync.dma_start(out=bt[:, :], in_=brows_d[l, i:i + 1, :])
                    brows.append(bt)
                b1c = _tile(wpool, [128, NM1], f32, "b1c")
                nc.sync.dma_start(out=b1c[:, :], in_=b1c_d[l])
                lncols = _tile(wpool, [128, 4 * NK], f32, "lncols")
                nc.sync.dma_start(out=lncols[:, :], in_=lnc_d[l])

                ctxT = emit_attention(nc, tc, pools, l, xT, wq, wk, wv, brows,
                                      consts)
                xres = emit_proj(nc, tc, pools, wo, l, ctxT, brows[3], "xres",
                                 resid=xT)
                xT = emit_layer_norm(nc, tc, pools, xres,
                                     lambda m: lncols[:, m:m + 1],
                                     lambda m: lncols[:, NK + m:NK + m + 1],
                                     consts, "xT")
                xres = emit_ffn(nc, tc, pools, l, xT, w1, w2, b1c, brows[4],
                                consts)
                xT = emit_layer_norm(nc, tc, pools, xres,
                                     lambda m: lncols[:, 2 * NK + m:2 * NK + m + 1],
                                     lambda m: lncols[:, 3 * NK + m:3 * NK + m + 1],
                                     consts, "xT")

            # ---- emission head: e[st] = x @ fc_w + fc_b   [128, 9] x 2
            e_tiles = []
            fcw_v = s_fcw[:, :].rearrange("p (k n) -> p k n", k=NK)
            for st in range(2):
                ps = _tile(psum, [128, N_TAGS], f32, "mm")
                for k in range(NK):
                    nc.tensor.matmul(ps[:, :],
                                     xT[k][:, 128 * st:128 * (st + 1)],
                                     fcw_v[:, k, :], start=(k == 0), stop=False)
                nc.tensor.matmul(ps[:, :], consts["ones_row"][0:1, 0:128],
                                 s_fcb[0:1, :], start=False, stop=True)
                e = _tile(acts, [128, N_TAGS], f32, "e")
                nc.vector.tensor_copy(e[:, :], ps[:, :])
                e_tiles.append(e)

            emit_viterbi(nc, tc, pools, e_tiles, consts, L, out_score,
                         out_path, fvd, bpd, featd)

    nc.compile()
    return nc


# ---------------------------------------------------------------- host side

def _prep_weights(inputs, L):
    """Pre-tile weights/constants into the device layouts. Returns dict of
    per-core-identical arrays."""
    g = lambda k: np.asarray(inputs[k], np.float32)

    def m_major(W, nmt):
        # W: [NL, K, M] -> [NL, nmt, 128, K] tiles
        nlk = W.shape[1]
        nkt = nlk // 128
        out = np.empty((NL, nmt, 128, nlk), np.float32)
        for l in range(NL):
            for m in range(nmt):
                blk = W[l][:, 128 * m:128 * (m + 1)]          # [K, 128]
                out[l, m] = blk.reshape(nkt, 128, 128).transpose(1, 0, 2).reshape(128, nlk)
        return out

    d = {}
    d["wq"] = m_major(g("Wq"), 6)
    d["wk"] = m_major(g("Wk"), 6)
    d["wv"] = np.ascontiguousarray(g("Wv"))
    d["wo"] = m_major(g("Wo"), 6)
    d["w1"] = m_major(g("W1"), NM1)
    d["w2"] = m_major(g("W2"), 6)
    d["brows"] = np.stack([g("bq"), g("bk"), g("bv"), g("bo"), g("b2")],
                          axis=1)  # [NL, 5, H]
    d["b1c"] = g("b1").reshape(NL, NM1, 128).transpose(0, 2, 1).copy()
    lnc = np.stack([g("ln1_s"), g("ln1_b"), g("ln2_s"), g("ln2_b")], axis=1)
    d["lnc"] = lnc.reshape(NL, 4, NK, 128).transpose(0, 3, 1, 2).reshape(
        NL, 128, 4 * NK).copy()
    eln = np.stack([g("emb_ln_s"), g("emb_ln_b")], axis=0)  # [2, H]
    d["elnc"] = eln.reshape(2, NK, 128).transpose(2, 0, 1).reshape(
        128, 2 * NK).copy()
    d["fcw"] = g("fc_w").reshape(NK, 128, N_TAGS).transpose(1, 0, 2).copy()
    d["fcb"] = g("fc_b").reshape(1, N_TAGS)

    tr = np.asarray(inputs["transitions"], np.float32)
    d["c_trrep"] = np.tile(tr.reshape(1, 81), (128, 1))
    revj = (9.0 - np.arange(9, dtype=np.float32))[None, :]        # [1,9] j
    d["c_revj"] = np.tile(np.tile(revj, (9, 1)).reshape(1, 81), (128, 1))
    d["c_ones"] = np.ones((128, 1), np.float32)
    d["c_onesrow"] = np.ones((1, 512), np.float32)
    misc = np.zeros((1, 128), np.float32)
    misc[0, 0:9] = np.arange(9)                                   # iota9
    fv0 = np.full(9, NEG, np.float32); fv0[START] = 0.0
    misc[0, 9:18] = fv0
    misc[0, 18:27] = tr[STOP]
    misc[0, 27:108] = tr.reshape(81)
    misc[0, 108] = EPS
    misc[0, 109:118] = 9.0 - np.arange(9)
    d["c_misc"] = misc
    return d


def kernel(**inputs):
    from concourse.bass_utils import run_bass_kernel_spmd

    L = int(np.asarray(inputs["targets_length"]))
    if L not in _CACHE:
        _CACHE[L] = build(L)
    nc = _CACHE[L]

    sentences = np.asarray(inputs["sentences"])
    we = np.asarray(inputs["word_emb"], np.float32)
    pe = np.asarray(inputs["pos_emb"], np.float32)
    te = np.asarray(inputs["type_emb"], np.float32)
    x0 = we[sentences] + pe[:S][None, :, :] + te[0][None, None, :]  # [B,S,H]

    shared = _prep_weights(inputs, L)
    in_maps = []
    for c in range(B):
        m = dict(shared)
        m["x0T"] = np.ascontiguousarray(x0[c].T)  # [H, S]
        in_maps.append(m)

    kwargs = {}
    if TRACE:
        import ntff_shim  # noqa
        import tempfile
        kwargs = dict(trace=True, tmpdir=tempfile.mkdtemp(prefix="bert_trace_"))
    res = run_bass_kernel_spmd(nc, in_maps, core_ids=list(range(B)), **kwargs)
    LAST["exec_time_ns"] = res.exec_time_ns
    LAST["results"] = res

    score = np.stack([res.results[c]["out_score"].reshape(()) for c in range(B)])
    path = np.stack([res.results[c]["out_path"].reshape(L) for c in range(B)])
    return score.astype(np.float32), path.astype(np.int32)
